# revision 10
# baseline (speedup 1.0000x reference)
"""NVFP4 QDQ linear layer on 8 Trainium2 NeuronCores.

Math (validated bit-exact vs the jax reference on the problem's data):
  qdqw = fp4_rne(w/(ws*ws2)) * (ws*ws2)        -> W' = qfp4*ws is bf16-exact
  qdqx = fp4_rne(x/(qs*s2)) * (qs*s2)          -> A  = qfp4*qs is bf16-exact
  out  = A @ W'^T * (s2*ws2) + bias            (bf16 matmul, fp32 accumulate)

fp4 round-to-nearest-even is computed with the magic-constant trick
(u + M) - M with M in {0.75*2^23, 1.5*2^23, 3*2^23} selected by |u| vs 2/4.
fp8(e4m3fn) RNE of qs is computed with exact integer bit arithmetic on DVE
(all adds kept < 2^21 so the fp32 ALU is exact).

Sharding: token-parallel. Each core processes 1024 tokens x full K, and
quantizes its own 512-row weight shard; W'^T shards are AllGather'd (bf16),
the global amax uses a tiny AllReduce-max. Output shards concatenate on the
token axis (no host compute).
"""
import sys

import numpy as np

for _p in ("/root/.axon_site/_ro/trn_rl_repo", "/opt/trn_rl_repo"):
    if _p not in sys.path:
        sys.path.append(_p)

B, S, K, O = 4, 2048, 4096, 4096
CORES = 8
T = B * S
TLOC = T // CORES      # 1024 tokens per core
OSH = O // CORES       # 512 out-features per core
BS = 16                # quant block size
NB = K // BS           # 256 blocks per row
CH = 512               # chain chunk (free dim)
NCH = K // CH          # 8 chunks per 128-row tile
XT = TLOC // 128       # 8 token row-tiles
WTT = OSH // 128       # 4 weight row-tiles
KT = K // 128          # 32 k tiles

R6 = float(np.float32(1.0) / np.float32(6.0))
R448 = float(np.float32(1.0) / np.float32(448.0))
CHALF = float(0.75 * 2**23)   # magic for grid step 0.5
CFULL = float(1.5 * 2**23)    # magic for grid step 1.0

_CACHE = {}


def _build():
    from concourse import bacc, tile, mybir, bass_isa, masks

    dt = mybir.dt
    Alu = mybir.AluOpType
    Act = mybir.ActivationFunctionType
    groups = [list(range(CORES))]

    nc = bacc.Bacc("TRN2", target_bir_lowering=False, debug=False, num_devices=CORES)

    x_in = nc.dram_tensor("x_shard", [TLOC, K], dt.float32, kind="ExternalInput")
    w_in = nc.dram_tensor("w_shard", [OSH, K], dt.float32, kind="ExternalInput")
    ws_in = nc.dram_tensor("ws_shard", [OSH, NB], dt.float32, kind="ExternalInput")
    ws2_in = nc.dram_tensor("ws2", [1, 1], dt.float32, kind="ExternalInput")
    b_in = nc.dram_tensor("bias_full", [1, O], dt.float32, kind="ExternalInput")
    out_d = nc.dram_tensor("out_shard", [TLOC, O], dt.float32, kind="ExternalOutput")

    wt_loc = nc.dram_tensor("wt_loc", [K, OSH], dt.bfloat16)
    wt_all = nc.dram_tensor("wt_all", [CORES, K, OSH], dt.bfloat16, addr_space="Shared")
    ar_in = nc.dram_tensor("ar_in", [1, 8], dt.float32)
    ar_out = nc.dram_tensor("ar_out", [1, 8], dt.float32, addr_space="Shared")
    amax_dram = nc.dram_tensor("amax_dram", [1, 128], dt.float32)
    vals_dram = nc.dram_tensor("vals_dram", [1, 4], dt.float32)

    with tile.TileContext(nc) as tc:
        with (
            tc.tile_pool(name="persist", bufs=1) as pp,
            tc.tile_pool(name="work", bufs=1) as wp,
            tc.tile_pool(name="psum", bufs=1, space="PSUM") as psp,
        ):
            # ---------- small setup ----------
            ident = pp.tile([128, 128], dt.bfloat16)
            masks.make_identity(nc, ident[:])
            ones_bf = pp.tile([1, 128], dt.bfloat16)
            nc.gpsimd.memset(ones_bf[:], 1.0)
            ones256 = pp.tile([128, NB], dt.float32)
            nc.gpsimd.memset(ones256[:], 1.0)

            ws2_t = pp.tile([1, 1], dt.float32)
            nc.sync.dma_start(ws2_t[:], ws2_in[:])
            ws2b = pp.tile([128, 1], dt.float32)
            nc.sync.dma_start(ws2b[:], ws2_in[0:1, :].broadcast_to([128, 1]))

            # persistent mediums
            pb_all = pp.tile([128, XT, NB], dt.float32)      # 8KB/part
            # A^T, one tensor per token row-tile (fine dep granularity)
            at_tiles = [
                pp.tile([128, KT, 128], dt.bfloat16, name=f"at_{tt}")
                for tt in range(XT)
            ]  # 8KB/part each, 64KB total

            # ---------- x phase 1: block-absmax (before amax collective) ----
            for tt in range(XT):
                for c in range(NCH):
                    xc = wp.tile([128, CH], dt.float32, tag="xc", bufs=4)
                    nc.sync.dma_start(
                        xc[:], x_in[tt * 128:(tt + 1) * 128, c * CH:(c + 1) * CH]
                    )
                    nc.vector.tensor_reduce(
                        pb_all[:, tt, c * (CH // BS):(c + 1) * (CH // BS)],
                        xc[:].rearrange("p (j i) -> p j i", i=BS),
                        axis=mybir.AxisListType.X,
                        op=Alu.max,
                        apply_absolute_value=True,
                    )

            # local amax: free-dim reduce, cross-partition via DRAM bounce,
            # then AllReduce(max) over the 8 cores.
            amax_loc = wp.tile([128, 1], dt.float32, tag="sm")
            nc.vector.tensor_reduce(
                amax_loc[:], pb_all[:], axis=mybir.AxisListType.XY, op=Alu.max
            )
            nc.sync.dma_start(amax_dram[0, :], amax_loc[:, 0])
            amax_row = wp.tile([1, 128], dt.float32, tag="sm2")
            nc.sync.dma_start(amax_row[:], amax_dram[:])
            amax_l = wp.tile([1, 1], dt.float32, tag="sm3")
            nc.vector.tensor_reduce(
                amax_l[:], amax_row[:], axis=mybir.AxisListType.X, op=Alu.max
            )
            amax8 = wp.tile([1, 8], dt.float32, tag="sm4")
            nc.vector.tensor_copy(amax8[:], amax_l[0:1, 0:1].broadcast_to([1, 8]))
            nc.sync.dma_start(ar_in[:], amax8[:])
            nc.gpsimd.collective_compute(
                "AllReduce", Alu.max, replica_groups=groups,
                ins=[ar_in[:]], outs=[ar_out[:]],
            )
            amax_g = pp.tile([1, 1], dt.float32)
            nc.sync.dma_start(amax_g[:], ar_out[0:1, 0:1])

            # scalars on partition 0: s2, 1/s2, alpha=s2*ws2, 1/alpha
            s2_1 = pp.tile([1, 1], dt.float32)
            nc.vector.tensor_scalar(s2_1[:], amax_g[:], R6, R448, Alu.mult, Alu.mult)
            rs2_1 = pp.tile([1, 1], dt.float32)
            nc.vector.reciprocal(rs2_1[:], s2_1[:])
            alpha_1 = pp.tile([1, 1], dt.float32)
            nc.vector.tensor_scalar(alpha_1[:], s2_1[:], ws2_t[:], None, Alu.mult)
            ralpha = pp.tile([1, 1], dt.float32)
            nc.vector.reciprocal(ralpha[:], alpha_1[:])
            vals = wp.tile([1, 4], dt.float32, tag="sm5")
            nc.vector.tensor_copy(vals[0:1, 0:1], s2_1[:])
            nc.vector.tensor_copy(vals[0:1, 1:2], rs2_1[:])
            nc.vector.tensor_copy(vals[0:1, 2:3], alpha_1[:])
            nc.vector.tensor_copy(vals[0:1, 3:4], alpha_1[:])
            nc.sync.dma_start(vals_dram[:], vals[:])
            valsb = pp.tile([128, 4], dt.float32)
            nc.sync.dma_start(valsb[:], vals_dram[0:1, :].broadcast_to([128, 4]))
            s2b = valsb[:, 0:1]
            rs2 = valsb[:, 1:2]
            alphab = valsb[:, 2:3]

            # ---------- shared fp4-QDQ chain ----------
            def chain(src_ap, rden_sl, scale_sl, out_bf):
                """out_bf = fp4_rne(src*rden) * scale  (all [128, CH])."""
                nj = CH // BS
                u = wp.tile([128, CH], dt.float32, tag="u", bufs=2)
                nc.vector.tensor_tensor(
                    u[:].rearrange("p (j i) -> p j i", i=BS),
                    src_ap.rearrange("p (j i) -> p j i", i=BS),
                    rden_sl.broadcast_to([128, nj, BS]),
                    Alu.mult,
                )
                au = wp.tile([128, CH], dt.float32, tag="au", bufs=2)
                nc.scalar.activation(au[:], u[:], Act.Abs)
                t2m = wp.tile([128, CH], dt.float32, tag="t2m", bufs=2)
                nc.vector.tensor_scalar(t2m[:], au[:], 2.0, CHALF, Alu.is_gt, Alu.mult)
                t4m = wp.tile([128, CH], dt.float32, tag="t4m", bufs=2)
                nc.vector.tensor_scalar(t4m[:], au[:], 4.0, CFULL, Alu.is_gt, Alu.mult)
                Mg = wp.tile([128, CH], dt.float32, tag="Mg", bufs=2)
                nc.vector.scalar_tensor_tensor(
                    Mg[:], t2m[:], CHALF, t4m[:], Alu.add, Alu.add
                )
                sg = wp.tile([128, CH], dt.float32, tag="sg", bufs=2)
                nc.vector.tensor_tensor(sg[:], u[:], Mg[:], Alu.add)
                rq = wp.tile([128, CH], dt.bfloat16, tag="rq", bufs=2)
                nc.vector.tensor_tensor(rq[:], sg[:], Mg[:], Alu.subtract)
                nc.vector.tensor_tensor(
                    out_bf[:].rearrange("p (j i) -> p j i", i=BS),
                    rq[:].rearrange("p (j i) -> p j i", i=BS),
                    scale_sl.broadcast_to([128, nj, BS]),
                    Alu.mult,
                )

            def transpose_blocks(src_bf, dst_fn):
                """PE-transpose the 4 [128,128] blocks of a [128, CH] bf16 tile."""
                for i in range(CH // 128):
                    tp = psp.tile([128, 128], dt.bfloat16, tag="tp", bufs=4)
                    nc.tensor.transpose(tp[:], src_bf[:, i * 128:(i + 1) * 128], ident[:])
                    dst_fn(i, tp)

            # ---------- weight pipeline (independent of amax) ----------
            for wt in range(WTT):
                ws_t = wp.tile([128, NB], dt.float32, tag="wssm", bufs=2)
                nc.sync.dma_start(ws_t[:], ws_in[wt * 128:(wt + 1) * 128, :])
                prodw = wp.tile([128, NB], dt.float32, tag="wssm2", bufs=2)
                nc.vector.tensor_scalar(prodw[:], ws_t[:], ws2b[:], None, Alu.mult)
                rinv_t = wp.tile([128, NB], dt.float32, tag="rinv", bufs=2)
                nc.vector.reciprocal(rinv_t[:], prodw[:])
                wsb_t = wp.tile([128, NB], dt.bfloat16, tag="wsb", bufs=2)
                nc.vector.tensor_copy(wsb_t[:], ws_t[:])
                for c in range(NCH):
                    wc = wp.tile([128, CH], dt.float32, tag="xc", bufs=4)
                    nc.sync.dma_start(
                        wc[:], w_in[wt * 128:(wt + 1) * 128, c * CH:(c + 1) * CH]
                    )
                    nj = CH // BS
                    wpc = wp.tile([128, CH], dt.bfloat16, tag="apc", bufs=2)
                    chain(
                        wc[:],
                        rinv_t[:, c * nj:(c + 1) * nj],
                        wsb_t[:, c * nj:(c + 1) * nj],
                        wpc,
                    )

                    def wt_dst(i, tp, c=c, wt=wt):
                        kb = c * (CH // 128) + i
                        stg = wp.tile([128, 128], dt.bfloat16, tag="wtstg", bufs=4)
                        nc.scalar.copy(stg[:], tp[:])
                        nc.sync.dma_start(
                            wt_loc[kb * 128:(kb + 1) * 128, wt * 128:(wt + 1) * 128],
                            stg[:],
                        )

                    transpose_blocks(wpc, wt_dst)

            nc.gpsimd.collective_compute(
                "AllGather", Alu.bypass, replica_groups=groups,
                ins=[wt_loc[:]], outs=[wt_all.ap()],
            )

            # ---------- x phase 2+3 ----------
            for tt in range(XT):
                pb = pb_all[:, tt, :]
                pbs = wp.tile([128, NB], dt.float32, tag="xsm", bufs=2)
                nc.vector.tensor_scalar(pbs[:], pb, R6, None, Alu.mult)
                qs0 = wp.tile([128, NB], dt.float32, tag="xsm2", bufs=2)
                nc.vector.tensor_scalar(qs0[:], pbs[:], rs2, None, Alu.mult)
                mz = wp.tile([128, NB], dt.uint8, tag="xsm3", bufs=2)
                nc.vector.tensor_scalar(mz[:], pb, 0.0, None, Alu.is_equal)
                nc.vector.copy_predicated(qs0[:], mz[:], ones256[:])
                # e4m3fn RNE via exact small-int arithmetic
                qu = qs0[:].bitcast(dt.uint32)
                lsbq = wp.tile([128, NB], dt.uint32, tag="bq1", bufs=2)
                nc.vector.tensor_scalar(lsbq[:], qu, 20, 1, Alu.logical_shift_right, Alu.bitwise_and)
                lowq = wp.tile([128, NB], dt.uint32, tag="bq2", bufs=2)
                nc.vector.tensor_scalar(lowq[:], qu, 0xFFFFF, None, Alu.bitwise_and)
                tq = wp.tile([128, NB], dt.uint32, tag="bq3", bufs=2)
                nc.vector.scalar_tensor_tensor(tq[:], lowq[:], float(0x7FFFF), lsbq[:], Alu.add, Alu.add)
                carry = wp.tile([128, NB], dt.uint32, tag="bq4", bufs=2)
                nc.vector.tensor_scalar(carry[:], tq[:], 0x100000, None, Alu.bitwise_and)
                highq = wp.tile([128, NB], dt.uint32, tag="bq5", bufs=2)
                nc.vector.tensor_scalar(highq[:], qu, 0xFFF00000, None, Alu.bitwise_and)
                qs_t = wp.tile([128, NB], dt.float32, tag="xsm4", bufs=2)
                nc.vector.tensor_tensor(qs_t[:].bitcast(dt.uint32), highq[:], carry[:], Alu.add)

                qsb_t = wp.tile([128, NB], dt.bfloat16, tag="qsb", bufs=2)
                nc.vector.tensor_copy(qsb_t[:], qs_t[:])
                den = wp.tile([128, NB], dt.float32, tag="xsm5", bufs=2)
                nc.vector.tensor_scalar(den[:], qs_t[:], s2b, None, Alu.mult)
                rden_t = wp.tile([128, NB], dt.float32, tag="rden", bufs=2)
                nc.vector.reciprocal(rden_t[:], den[:])

                for c in range(NCH):
                    xc2 = wp.tile([128, CH], dt.float32, tag="xc", bufs=4)
                    nc.sync.dma_start(
                        xc2[:], x_in[tt * 128:(tt + 1) * 128, c * CH:(c + 1) * CH]
                    )
                    nj = CH // BS
                    apc = wp.tile([128, CH], dt.bfloat16, tag="apc", bufs=2)
                    chain(
                        xc2[:],
                        rden_t[:, c * nj:(c + 1) * nj],
                        qsb_t[:, c * nj:(c + 1) * nj],
                        apc,
                    )

                    def at_dst(i, tp, c=c, tt=tt):
                        kb = c * (CH // 128) + i
                        nc.scalar.copy(at_tiles[tt][:, kb, :], tp[:])

                    transpose_blocks(apc, at_dst)

            # ---------- matmul ----------
            for ot in range(CORES):
                bseg = wp.tile([1, OSH], dt.float32, tag="bseg", bufs=2)
                nc.sync.dma_start(bseg[:], b_in[0:1, ot * OSH:(ot + 1) * OSH])
                bpseg = wp.tile([1, OSH], dt.bfloat16, tag="bpseg", bufs=2)
                nc.vector.tensor_scalar(bpseg[:], bseg[:], ralpha[:], None, Alu.mult)
                halves = []
                for h in range(2):
                    wth = wp.tile([128, 16, OSH], dt.bfloat16, tag="wth", bufs=3)
                    for k2 in range(16):
                        kt = h * 16 + k2
                        nc.sync.dma_start(
                            wth[:, k2, :], wt_all[ot, kt * 128:(kt + 1) * 128, :]
                        )
                    halves.append(wth)
                for tt in range(XT):
                    mmps = psp.tile([128, OSH], dt.float32, tag="mm", bufs=3)
                    for kt in range(KT):
                        nc.tensor.matmul(
                            mmps[:],
                            at_tiles[tt][:, kt, :],
                            halves[kt // 16][:, kt % 16, :],
                            start=(kt == 0),
                            stop=False,
                        )
                    nc.tensor.matmul(
                        mmps[:], ones_bf[:],
                        bpseg[:],
                        start=False, stop=True,
                    )
                    osb = wp.tile([128, OSH], dt.float32, tag="osb", bufs=3)
                    nc.scalar.activation(
                        osb[:], mmps[:], Act.Copy, bias=0.0, scale=alphab
                    )
                    nc.sync.dma_start(
                        out_d[tt * 128:(tt + 1) * 128, ot * OSH:(ot + 1) * OSH],
                        osb[:],
                    )

    nc.compile()
    return nc


def _get_nc():
    if "nc" not in _CACHE:
        _CACHE["nc"] = _build()
    return _CACHE["nc"]


def kernel(x, weight, weight_scale, weight_scale_2, bias, block_size):
    from concourse.bass_utils import run_bass_kernel_spmd

    assert int(block_size) == BS
    x = np.ascontiguousarray(np.asarray(x, np.float32)).reshape(T, K)
    weight = np.ascontiguousarray(np.asarray(weight, np.float32))
    weight_scale = np.ascontiguousarray(np.asarray(weight_scale, np.float32))
    ws2 = np.asarray(weight_scale_2, np.float32).reshape(1, 1)
    bias2 = np.ascontiguousarray(np.asarray(bias, np.float32)).reshape(1, O)

    nc = _get_nc()
    in_maps = [
        {
            "x_shard": x[r * TLOC:(r + 1) * TLOC],
            "w_shard": weight[r * OSH:(r + 1) * OSH],
            "ws_shard": weight_scale[r * OSH:(r + 1) * OSH],
            "ws2": ws2,
            "bias_full": bias2,
        }
        for r in range(CORES)
    ]
    res = run_bass_kernel_spmd(nc, in_maps, list(range(CORES)), trace=False)
    out = np.concatenate(
        [res.results[r]["out_shard"] for r in range(CORES)], axis=0
    )
    return out.reshape(B, S, O).astype(np.float32)
